# revision 18
# baseline (speedup 1.0000x reference)
"""Trainium2 Bass kernel for nn_ASTDecoder (4-layer transformer decoder,
B=4, S=M=1024, D=512, H=8, DFF=2048, fp32).

Sharding: data-parallel over batch. Core c computes batch element c%4 end to
end (cores 4-7 are duplicates whose outputs are ignored).

On-core layout: activations live feature-major ("transposed", [D, tok]) so
every projection is matmul(lhsT=W[D,out], rhs=actT) with natural weights.
Attention computes transposed scores s^T[k,q] = matmul(lhsT=k^T, rhs=q^T);
softmax skips the max-subtraction (scores are O(1) for this model), the
denominator comes for free from a ones-column appended to V, and the causal
mask is applied as a column-range restriction plus one triangular block
multiply per diagonal tile. All matmuls run in float32r.
"""

import sys

sys.path.insert(0, '/opt/trn_rl_repo')

import ml_dtypes
import numpy as np

import concourse.bass as bass  # noqa: F401
import concourse.tile as tile
import concourse.mybir as mybir
from concourse import bacc
from concourse.bass_utils import run_bass_kernel_spmd

F32 = mybir.dt.float32
F32R = mybir.dt.float32r
BF16 = mybir.dt.bfloat16
AF = mybir.ActivationFunctionType
ALU = mybir.AluOpType

B, S, M, D, H, L, DFF = 4, 1024, 1024, 512, 8, 4, 2048
DK = D // H          # 64
NT = D // 128        # 4 feature tiles
TT = S // 128        # 8 token tiles
QH = S // 512        # 2 query halves

_cache = {}

_tables_patched = False


def _patch_act_tables():
    """Route Exp and Ln to the combined natural_log_exp set so LayerNorm's
    Ln->Exp rstd and the attention exps share one ACT table (avoids ~2.7us
    table reloads inside every LayerNorm critical path)."""
    global _tables_patched
    if _tables_patched:
        return
    import concourse.hw_specs as hw_specs
    orig = hw_specs.get_activation_tables

    def patched(module_arch):
        t = {k: set(v) for k, v in orig(module_arch).items()}
        exp = mybir.ActivationFunctionType.Exp
        ln = mybir.ActivationFunctionType.Ln
        for name, funcs in t.items():
            if name != "natural_log_exp_and_others":
                funcs.discard(exp)
                funcs.discard(ln)
        return t

    hw_specs.get_activation_tables = patched
    bacc.get_activation_tables = patched
    _tables_patched = True


def build_program(num_layers=L):
    _patch_act_tables()
    nc = bacc.Bacc("TRN2", target_bir_lowering=False, debug=False, num_devices=8)
    nl = num_layers

    # ---- DRAM I/O ----
    xT_d = nc.dram_tensor("xT", [D, S], F32R, kind="ExternalInput")
    lmemT_d = nc.dram_tensor("lmemT", [D, M], BF16, kind="ExternalInput")
    rmemT_d = nc.dram_tensor("rmemT", [D, M], BF16, kind="ExternalInput")
    tri_d = nc.dram_tensor("tri", [128, 128], BF16, kind="ExternalInput")

    def wt(name, shape):
        return nc.dram_tensor(name, shape, BF16, kind="ExternalInput")

    def ft(name, shape):
        return nc.dram_tensor(name, shape, F32, kind="ExternalInput")

    w_d = {}
    for a in ("s", "l", "r"):
        for m_ in ("q", "k", "v", "o"):
            w_d[m_ + a] = wt(f"w{m_}{a}", [nl, D, D])
    w_d["1"] = wt("w1", [nl, D, DFF])
    w_d["2"] = wt("w2", [nl, DFF, D])

    b_d = {}
    for a in ("s", "l", "r"):
        for m_ in ("q", "k", "o"):
            b_d[m_ + a] = ft(f"b{m_}{a}", [nl, 128, NT])
        b_d["v" + a] = ft(f"bv{a}", [nl, 1, D])
    b_d["1"] = ft("b1", [nl, 128, DFF // 128])
    b_d["2"] = ft("b2", [nl, 128, NT])
    lns_d = ft("lns", [nl, 4, 128, NT])
    lnb_d = ft("lnb", [nl, 4, 128, NT])
    fns_d = ft("fns", [1, 128, NT])
    fnb_d = ft("fnb", [1, 128, NT])

    out_d = nc.dram_tensor("out", [D, S], F32, kind="ExternalOutput")

    with tile.TileContext(nc) as tc:
        import contextlib
        with contextlib.ExitStack() as ctx:
            big = ctx.enter_context(tc.tile_pool(name="big", bufs=1))
            wpool = ctx.enter_context(tc.tile_pool(name="w", bufs=7))
            epool = ctx.enter_context(tc.tile_pool(name="e", bufs=3))
            scr1 = ctx.enter_context(tc.tile_pool(name="scr1", bufs=1))
            scr2 = ctx.enter_context(tc.tile_pool(name="scr2", bufs=2))
            consts = ctx.enter_context(tc.tile_pool(name="consts", bufs=1))
            lparam = ctx.enter_context(tc.tile_pool(name="lparam", bufs=2))
            ps_mm = ctx.enter_context(tc.tile_pool(name="psmm", bufs=2, space="PSUM"))
            ps_av = ctx.enter_context(tc.tile_pool(name="psav", bufs=2, space="PSUM"))
            ps_sc = ctx.enter_context(tc.tile_pool(name="pssc", bufs=2, space="PSUM"))
            avs_p = ctx.enter_context(tc.tile_pool(name="avs", bufs=4))

            # ---- persistent tiles ----
            xt = big.tile([128, NT, S], F32R, tag="x")
            lmem = big.tile([128, NT, M], BF16, tag="lmem")
            rmem = big.tile([128, NT, M], BF16, tag="rmem")
            # act pool: per-sublayer tiles; same-tag allocations reuse the
            # slot serially (ht and at share "ha": never live simultaneously).
            act = ctx.enter_context(tc.tile_pool(name="act", bufs=1))
            kv = ctx.enter_context(tc.tile_pool(name="kv", bufs=3))

            nc.sync.dma_start(out=xt, in_=xT_d.rearrange("(t p) s -> p t s", p=128))
            nc.sync.dma_start(out=lmem, in_=lmemT_d.rearrange("(t p) s -> p t s", p=128))
            nc.sync.dma_start(out=rmem, in_=rmemT_d.rearrange("(t p) s -> p t s", p=128))

            tri = consts.tile([128, 128], BF16)
            nc.sync.dma_start(out=tri, in_=tri_d[:])
            ones_f = consts.tile([128, 64], F32)
            nc.vector.memset(ones_f, 1.0)
            ones_r = consts.tile([128, 1], F32R)
            nc.vector.tensor_copy(out=ones_r, in_=ones_f[:, 0:1])
            zcol = consts.tile([128, 1], F32)
            nc.vector.memset(zcol, 0.0)
            epst = consts.tile([1, 1], F32)
            nc.vector.memset(epst, 1e-5)

            def load_w(dram_ap):
                t = wpool.tile([128, 4, 512], BF16, tag="w")
                nc.sync.dma_start(out=t, in_=dram_ap)
                return t

            def w_slice(dram, l_, q=None):
                # dram [nl, IN, OUT] -> [128, 4, 512] AP
                a = dram[l_].rearrange("(t p) n -> p t n", p=128)
                if q is not None:  # quarter of the free dim
                    a = a[:, :, q * 512:(q + 1) * 512]
                return a

            def emit_ln(x_in, s_ap, b_ap, out_t, out_dtype_is_f32=False):
                """out_t[:, t, :] = (x - mu)/sqrt(var+eps) * s[t] + b[t].
                s_ap/b_ap: [128, NT] SBUF APs."""
                for qh in range(QH):
                    sl = slice(qh * 512, (qh + 1) * 512)
                    sum1 = ps_mm.tile([1, 512], F32, tag="mm", name="sum1")
                    sum2 = ps_mm.tile([1, 512], F32, tag="mm", name="sum2")
                    for k in range(NT):
                        nc.tensor.matmul(sum1, ones_r, x_in[:, k, sl],
                                         start=(k == 0), stop=(k == NT - 1))
                    for k in range(NT):
                        sqt = scr2.tile([128, 512], F32R, tag="sq")
                        nc.vector.tensor_mul(sqt, x_in[:, k, sl], x_in[:, k, sl])
                        nc.tensor.matmul(sum2, ones_r, sqt,
                                         start=(k == 0), stop=(k == NT - 1))
                    mu = scr1.tile([1, 512], F32, tag="mu")
                    nc.vector.tensor_scalar_mul(mu, sum1, 1.0 / D)
                    mm = scr1.tile([1, 512], F32, tag="mm2")
                    nc.vector.tensor_mul(mm, mu, mu)
                    var = scr1.tile([1, 512], F32, tag="var")
                    nc.vector.scalar_tensor_tensor(
                        out=var, in0=sum2, scalar=1.0 / D, in1=mm,
                        op0=ALU.mult, op1=ALU.subtract)
                    rstd = scr1.tile([1, 512], F32, tag="rstd")
                    nc.scalar.activation(rstd, var, AF.Ln, bias=epst)
                    nc.scalar.activation(rstd, rstd, AF.Exp, scale=-0.5)
                    mub = scr1.tile([128, 512], F32, tag="mub")
                    rstdb = scr1.tile([128, 512], F32, tag="rstdb")
                    nc.gpsimd.partition_broadcast(mub, mu)
                    nc.gpsimd.partition_broadcast(rstdb, rstd)
                    for k in range(NT):
                        tmp = scr1.tile([128, 512], F32, tag="lntmp")
                        nc.vector.tensor_sub(tmp, x_in[:, k, sl], mub)
                        nc.vector.tensor_mul(tmp, tmp, rstdb)
                        nc.vector.tensor_scalar(
                            out=out_t[:, k, sl], in0=tmp,
                            scalar1=s_ap[:, k:k + 1], scalar2=b_ap[:, k:k + 1],
                            op0=ALU.mult, op1=ALU.add)

            def emit_projT(w_sb, rhs_t, out_t, bias_sb):
                """out_t[:, m, :] (f32r, [128,NT,S]) = W^T @ rhs + bias.
                w_sb [128,4,512], rhs_t [128,NT,S] f32r, bias_sb [128,NT]."""
                for m_ in range(NT):
                    for qh in range(QH):
                        sl = slice(qh * 512, (qh + 1) * 512)
                        ps = ps_mm.tile([128, 512], F32, tag="mm")
                        for k in range(NT):
                            nc.tensor.matmul(
                                ps, w_sb[:, k, m_ * 128:(m_ + 1) * 128],
                                rhs_t[:, k, sl],
                                start=(k == 0), stop=(k == NT - 1))
                        nc.vector.tensor_scalar(
                            out=out_t[:, m_, sl], in0=ps,
                            scalar1=bias_sb[:, m_:m_ + 1], scalar2=None,
                            op0=ALU.add)

            def emit_v(w_sb, src_t, bvb, vt):
                """vt[:, mt, h, 0:DK] = (src^T)^T @ Wv + bv (natural layout)."""
                nc.vector.tensor_copy(
                    out=vt[:, :, :, DK:DK + 1].rearrange("p a b c -> p (a b c)"),
                    in_=ones_f)
                for mt in range(TT):
                    ps = ps_mm.tile([128, 512], F32, tag="mm")
                    for k in range(NT):
                        nc.tensor.matmul(
                            ps, src_t[:, k, mt * 128:(mt + 1) * 128],
                            w_sb[:, k, :],
                            start=(k == 0), stop=(k == NT - 1))
                    nc.vector.tensor_tensor(
                        out=vt[:, mt, :, 0:DK],
                        in0=ps.rearrange("p (h d) -> p h d", h=H),
                        in1=bvb.rearrange("p (h d) -> p h d", h=H),
                        op=ALU.add)

            def emit_attn(is_self, qt, kt_t, vt, at):
                """at = softmax(k^T q / sqrt(dk)) V, all transposed layouts.
                Heads run in even/odd pairs: their score matmuls use PE row
                groups 0-63 / 64-127 (tile_position auto-derived from the
                lhsT base partition) and execute concurrently."""
                for qh in range(QH):
                    ktmax = (qh + 1) * 4 if is_self else TT
                    for hp in range(H // 2):
                        ets = []
                        for sub in range(2):
                            ets.append(epool.tile([128, TT, 512], BF16, tag="e",
                                                  name=f"et{sub}"))
                        # interleave the two heads' score matmuls; each
                        # sc tile holds 2 k-tiles (2 PSUM banks) so one exp
                        # call covers both.
                        for kg in range((ktmax + 1) // 2):
                            kts = [k for k in (2 * kg, 2 * kg + 1) if k < ktmax]
                            scs = []
                            for sub in range(2):
                                scs.append(ps_sc.tile([128, 2, 512], F32,
                                                      tag="sc", name=f"sc{sub}"))
                            for j, k in enumerate(kts):
                                c0 = max(0, k - qh * 4) * 128 if is_self else 0
                                for sub in range(2):
                                    h_ = 2 * hp + sub
                                    po = (h_ % 2) * 64
                                    ft_ = h_ // 2
                                    nc.tensor.matmul(
                                        scs[sub][:, j, c0:],
                                        kt_t[po:po + 64, ft_, k * 128:(k + 1) * 128],
                                        qt[po:po + 64, ft_, qh * 512 + c0:(qh + 1) * 512],
                                        start=True, stop=True)
                            cmin = (max(0, 2 * kg - qh * 4) * 128
                                    if is_self else 0)
                            for sub in range(2):
                                nc.scalar.activation(
                                    ets[sub][:, 2 * kg:2 * kg + len(kts), cmin:],
                                    scs[sub][:, 0:len(kts), cmin:], AF.Exp,
                                    scale=1.0 / np.sqrt(DK))
                                if is_self:
                                    for k in kts:
                                        if k >= qh * 4:
                                            c0 = (k - qh * 4) * 128
                                            nc.gpsimd.tensor_mul(
                                                ets[sub][:, k, c0:c0 + 128],
                                                ets[sub][:, k, c0:c0 + 128], tri)
                        for sub in range(2):
                            h_ = 2 * hp + sub
                            po = (h_ % 2) * 64
                            ft_ = h_ // 2
                            et = ets[sub]
                            av = ps_av.tile([DK + 1, 512], F32, tag="av")
                            for k in range(ktmax):
                                c0 = max(0, k - qh * 4) * 128 if is_self else 0
                                nc.tensor.matmul(
                                    av[:, c0:], vt[:, k, h_, :], et[:, k, c0:],
                                    start=(k == 0), stop=(k == ktmax - 1))
                            # move the AV result to SBUF right away so the
                            # PSUM slot frees; normalize off the critical path
                            sb65 = avs_p.tile([DK + 1, 512], F32, tag="avs")
                            nc.vector.tensor_copy(out=sb65, in_=av)
                            rds = scr1.tile([1, 512], F32, tag="rds")
                            nc.vector.tensor_copy(out=rds, in_=sb65[DK:DK + 1, :])
                            rd = scr1.tile([1, 512], F32, tag="rd")
                            nc.vector.reciprocal_approx_fast(out=rd, in_=rds)
                            rdb = scr2.tile([64, 512], F32, tag="rdb")
                            nc.gpsimd.partition_broadcast(rdb, rd)
                            nc.vector.tensor_mul(
                                at[po:po + 64, ft_, qh * 512:(qh + 1) * 512],
                                sb65[0:DK, :], rdb)

            def emit_resid(w_sb, rhs_t, bias_sb):
                """x += W^T @ rhs + bias (out-projection / FFN-2 path)."""
                for m_ in range(NT):
                    for qh in range(QH):
                        sl = slice(qh * 512, (qh + 1) * 512)
                        ps = ps_mm.tile([128, 512], F32, tag="mm")
                        for k in range(NT):
                            nc.tensor.matmul(
                                ps, w_sb[:, k, m_ * 128:(m_ + 1) * 128],
                                rhs_t[:, k, sl],
                                start=(k == 0), stop=(k == NT - 1))
                        nc.vector.scalar_tensor_tensor(
                            out=xt[:, m_, sl], in0=ps,
                            scalar=bias_sb[:, m_:m_ + 1], in1=xt[:, m_, sl],
                            op0=ALU.add, op1=ALU.add)

            def load_bias_pp(dram, l_):
                t = lparam.tile([128, NT], F32, tag="bpp")
                nc.sync.dma_start(out=t, in_=dram[l_])
                return t

            for l_ in range(nl):
                lns = lparam.tile([128, 4, NT], F32, tag="lns")
                lnb = lparam.tile([128, 4, NT], F32, tag="lnb")
                nc.sync.dma_start(out=lns, in_=lns_d[l_].rearrange("a p t -> p a t"))
                nc.sync.dma_start(out=lnb, in_=lnb_d[l_].rearrange("a p t -> p a t"))

                # -- self QKV --
                ht = act.tile([128, NT, S], BF16, tag="ha")
                emit_ln(xt, lns[:, 0, :], lnb[:, 0, :], ht)
                kvt = {}

                def qkv(a, src_t, with_q):
                    wk = load_w(w_slice(w_d["k" + a], l_))
                    wv = load_w(w_slice(w_d["v" + a], l_))
                    bk = load_bias_pp(b_d["k" + a], l_)
                    bvr = scr1.tile([1, D], F32, tag="bvr")
                    nc.sync.dma_start(out=bvr, in_=b_d["v" + a][l_])
                    bvb = scr1.tile([128, D], F32, tag="bvb")
                    nc.gpsimd.partition_broadcast(bvb, bvr)
                    kt_t = kv.tile([128, NT, S], BF16, tag="kt")
                    vt = kv.tile([128, TT, H, DK + 1], BF16, tag="vt")
                    emit_projT(wk, src_t, kt_t, bk)
                    emit_v(wv, src_t, bvb, vt)
                    kvt[a] = (kt_t, vt)

                def qproj(a, ht_):
                    wq = load_w(w_slice(w_d["q" + a], l_))
                    bq = load_bias_pp(b_d["q" + a], l_)
                    qt = act.tile([128, NT, S], BF16, tag="qt")
                    emit_projT(wq, ht_, qt, bq)
                    return qt

                def oproj(a, at_):
                    wo = load_w(w_slice(w_d["o" + a], l_))
                    bo = load_bias_pp(b_d["o" + a], l_)
                    emit_resid(wo, at_, bo)

                qt = qproj("s", ht)
                qkv("s", ht, True)
                at = act.tile([128, NT, S], BF16, tag="ha")
                emit_attn(True, qt, kvt["s"][0], kvt["s"][1], at)
                oproj("s", at)
                # cross K/V: no dependency on x -- emitted after self-attn so
                # the scheduler uses them to fill PE gaps (LN chains etc.)
                qkv("l", lmem, False)
                qkv("r", rmem, False)

                for si, a in ((1, "l"), (2, "r")):
                    ht = act.tile([128, NT, S], BF16, tag="ha")
                    emit_ln(xt, lns[:, si, :], lnb[:, si, :], ht)
                    qt = qproj(a, ht)
                    at = act.tile([128, NT, S], BF16, tag="ha")
                    emit_attn(False, qt, kvt[a][0], kvt[a][1], at)
                    oproj(a, at)

                # ---- FFN ----
                ht = act.tile([128, NT, S], BF16, tag="ha")
                emit_ln(xt, lns[:, 3, :], lnb[:, 3, :], ht)
                b1 = lparam.tile([128, DFF // 128], F32, tag="b1")
                nc.sync.dma_start(out=b1, in_=b_d["1"][l_])
                b2 = load_bias_pp(b_d["2"], l_)
                for qr in range(4):
                    h1 = act.tile([128, 4, S], BF16, tag="h1")
                    w1 = load_w(w_slice(w_d["1"], l_, q=qr))
                    w2 = load_w(
                        w_d["2"][l_].rearrange("(t p) n -> p t n", p=128)
                        [:, qr * 4:(qr + 1) * 4, :])
                    for dt_ in range(4):
                        for qh in range(QH):
                            sl = slice(qh * 512, (qh + 1) * 512)
                            ps = ps_mm.tile([128, 512], F32, tag="mm")
                            for k in range(NT):
                                nc.tensor.matmul(
                                    ps, w1[:, k, dt_ * 128:(dt_ + 1) * 128],
                                    ht[:, k, sl],
                                    start=(k == 0), stop=(k == NT - 1))
                            nc.scalar.activation(
                                h1[:, dt_, sl], ps, AF.Gelu_apprx_tanh,
                                bias=b1[:, qr * 4 + dt_:qr * 4 + dt_ + 1])
                    for m_ in range(NT):
                        for qh in range(QH):
                            sl = slice(qh * 512, (qh + 1) * 512)
                            ps = ps_mm.tile([128, 512], F32, tag="mm")
                            for dt_ in range(4):
                                nc.tensor.matmul(
                                    ps, w2[:, dt_, m_ * 128:(m_ + 1) * 128],
                                    h1[:, dt_, sl],
                                    start=(dt_ == 0), stop=(dt_ == 3))
                            bsl = b2[:, m_:m_ + 1] if qr == 0 else zcol
                            nc.vector.scalar_tensor_tensor(
                                out=xt[:, m_, sl], in0=ps, scalar=bsl,
                                in1=xt[:, m_, sl], op0=ALU.add, op1=ALU.add)

            # ---- final LN + output ----
            fns = lparam.tile([128, NT], F32, tag="fns")
            fnb = lparam.tile([128, NT], F32, tag="fnb")
            nc.sync.dma_start(out=fns, in_=fns_d[0])
            nc.sync.dma_start(out=fnb, in_=fnb_d[0])
            outt = act.tile([128, NT, S], F32, tag="qt")
            emit_ln(xt, fns, fnb, outt, out_dtype_is_f32=True)
            nc.sync.dma_start(out=out_d.rearrange("(t p) s -> p t s", p=128),
                              in_=outt)

    nc.compile()
    return nc


def _prep_inputs(inputs, num_layers=L):
    """Build per-core in_maps from the full problem inputs."""
    nl = num_layers
    f32 = np.float32
    g = {k: np.asarray(v, dtype=f32) if np.asarray(v).dtype != np.bool_ else v
         for k, v in inputs.items()}

    def pp(a):  # [nl, D] -> [nl, 128, NT] per-partition layout
        return np.ascontiguousarray(
            a[:nl].reshape(nl, NT, 128).transpose(0, 2, 1))

    tri = np.tril(np.ones((128, 128), f32)).T  # tri[p, j] = 1 if p <= j
    common = {}
    for i, a in enumerate(("s", "l", "r")):
        wqkv = g["Wqkv_self" if a == "s" else f"Wqkv_{a}"][:nl]
        bqkv = g["bqkv_self" if a == "s" else f"bqkv_{a}"][:nl]
        wo = g["Wo_self" if a == "s" else f"Wo_{a}"][:nl]
        bo = g["bo_self" if a == "s" else f"bo_{a}"][:nl]
        common[f"wq{a}"] = np.ascontiguousarray(wqkv[:, 0]).astype(ml_dtypes.bfloat16)
        common[f"wk{a}"] = np.ascontiguousarray(wqkv[:, 1]).astype(ml_dtypes.bfloat16)
        common[f"wv{a}"] = np.ascontiguousarray(wqkv[:, 2]).astype(ml_dtypes.bfloat16)
        common[f"wo{a}"] = np.ascontiguousarray(wo).astype(ml_dtypes.bfloat16)
        common[f"bq{a}"] = pp(bqkv[:, 0])
        common[f"bk{a}"] = pp(bqkv[:, 1])
        common[f"bv{a}"] = np.ascontiguousarray(bqkv[:, 2]).reshape(nl, 1, D)
        common[f"bo{a}"] = pp(bo)
    common["w1"] = np.ascontiguousarray(g["W1"][:nl]).astype(ml_dtypes.bfloat16)
    common["w2"] = np.ascontiguousarray(g["W2"][:nl]).astype(ml_dtypes.bfloat16)
    common["b1"] = np.ascontiguousarray(
        g["b1"][:nl].reshape(nl, DFF // 128, 128).transpose(0, 2, 1))
    common["b2"] = pp(g["b2"][:nl])
    common["lns"] = np.ascontiguousarray(
        g["ln_scale"][:nl].reshape(nl, 4, NT, 128).transpose(0, 1, 3, 2))
    common["lnb"] = np.ascontiguousarray(
        g["ln_bias"][:nl].reshape(nl, 4, NT, 128).transpose(0, 1, 3, 2))
    common["fns"] = g["fnorm_scale"].reshape(1, NT, 128).transpose(0, 2, 1).copy()
    common["fnb"] = g["fnorm_bias"].reshape(1, NT, 128).transpose(0, 2, 1).copy()
    common["tri"] = tri.astype(ml_dtypes.bfloat16)

    in_maps = []
    for c in range(8):
        b = c % B
        m = dict(common)
        m["xT"] = np.ascontiguousarray(g["tgt_emb"][b].T)
        m["lmemT"] = np.ascontiguousarray(g["l_mem_emb"][b].T).astype(ml_dtypes.bfloat16)
        m["rmemT"] = np.ascontiguousarray(g["r_mem_emb"][b].T).astype(ml_dtypes.bfloat16)
        in_maps.append(m)
    return in_maps


def run(inputs, num_layers=L, trace=False, tmpdir=None):
    key = num_layers
    if key not in _cache:
        _cache[key] = build_program(num_layers)
    nc = _cache[key]
    in_maps = _prep_inputs(inputs, num_layers)
    res = run_bass_kernel_spmd(nc, in_maps, core_ids=list(range(8)),
                               trace=trace, tmpdir=tmpdir)
    out = np.stack([res.results[b]["out"].T for b in range(B)])
    return out, res


def kernel(**inputs):
    out, _ = run(inputs)
    return out.astype(np.float32)


# revision 19
# speedup vs baseline: 1.2524x; 1.2524x over previous
"""Trainium2 Bass kernel for nn_ASTDecoder (4-layer transformer decoder,
B=4, S=M=1024, D=512, H=8, DFF=2048, fp32).

Sharding: data-parallel over batch. Core c computes batch element c%4 end to
end (cores 4-7 are duplicates whose outputs are ignored).

On-core layout: activations live feature-major ("transposed", [D, tok]) so
every projection is matmul(lhsT=W[D,out], rhs=actT) with natural weights.
Attention computes transposed scores s^T[k,q] = matmul(lhsT=k^T, rhs=q^T);
softmax skips the max-subtraction (scores are O(1) for this model), the
denominator comes for free from a ones-column appended to V, and the causal
mask is applied as a column-range restriction plus one triangular block
multiply per diagonal tile. All matmuls run in float32r.
"""

import sys

sys.path.insert(0, '/opt/trn_rl_repo')

import ml_dtypes
import numpy as np

import concourse.bass as bass  # noqa: F401
import concourse.tile as tile
import concourse.mybir as mybir
from concourse import bacc
from concourse.bass_utils import run_bass_kernel_spmd

F32 = mybir.dt.float32
F32R = mybir.dt.float32r
BF16 = mybir.dt.bfloat16
AF = mybir.ActivationFunctionType
ALU = mybir.AluOpType

B, S, M, D, H, L, DFF = 4, 1024, 1024, 512, 8, 4, 2048
DK = D // H          # 64
NT = D // 128        # 4 feature tiles
TT = S // 128        # 8 token tiles
QH = S // 512        # 2 query halves

_cache = {}

_tables_patched = False


def _patch_act_tables():
    """Route Exp and Ln to the combined natural_log_exp set so LayerNorm's
    Ln->Exp rstd and the attention exps share one ACT table (avoids ~2.7us
    table reloads inside every LayerNorm critical path)."""
    global _tables_patched
    if _tables_patched:
        return
    import concourse.hw_specs as hw_specs
    orig = hw_specs.get_activation_tables

    def patched(module_arch):
        t = {k: set(v) for k, v in orig(module_arch).items()}
        exp = mybir.ActivationFunctionType.Exp
        ln = mybir.ActivationFunctionType.Ln
        for name, funcs in t.items():
            if name != "natural_log_exp_and_others":
                funcs.discard(exp)
                funcs.discard(ln)
        return t

    hw_specs.get_activation_tables = patched
    bacc.get_activation_tables = patched
    _tables_patched = True


def build_program(num_layers=L):
    _patch_act_tables()
    nc = bacc.Bacc("TRN2", target_bir_lowering=False, debug=False, num_devices=8)
    nl = num_layers

    # ---- DRAM I/O ----
    xT_d = nc.dram_tensor("xT", [D, S], F32R, kind="ExternalInput")
    lmemT_d = nc.dram_tensor("lmemT", [D, M], BF16, kind="ExternalInput")
    rmemT_d = nc.dram_tensor("rmemT", [D, M], BF16, kind="ExternalInput")
    tri_d = nc.dram_tensor("tri", [128, 128], BF16, kind="ExternalInput")

    def wt(name, shape):
        return nc.dram_tensor(name, shape, BF16, kind="ExternalInput")

    def ft(name, shape):
        return nc.dram_tensor(name, shape, F32, kind="ExternalInput")

    w_d = {}
    for a in ("s", "l", "r"):
        for m_ in ("q", "k", "v", "o"):
            w_d[m_ + a] = wt(f"w{m_}{a}", [nl, D, D])
    w_d["1"] = wt("w1", [nl, D, DFF])
    w_d["2"] = wt("w2", [nl, DFF, D])

    b_d = {}
    for a in ("s", "l", "r"):
        for m_ in ("q", "k", "o"):
            b_d[m_ + a] = ft(f"b{m_}{a}", [nl, 128, NT])
        b_d["v" + a] = ft(f"bv{a}", [nl, 1, D])
    b_d["1"] = ft("b1", [nl, 128, DFF // 128])
    b_d["2"] = ft("b2", [nl, 128, NT])
    lns_d = ft("lns", [nl, 4, 128, NT])
    lnb_d = ft("lnb", [nl, 4, 128, NT])
    fns_d = ft("fns", [1, 128, NT])
    fnb_d = ft("fnb", [1, 128, NT])

    out_d = nc.dram_tensor("out", [D, S], F32, kind="ExternalOutput")

    with tile.TileContext(nc) as tc:
        import contextlib
        with contextlib.ExitStack() as ctx:
            big = ctx.enter_context(tc.tile_pool(name="big", bufs=1))
            wpool = ctx.enter_context(tc.tile_pool(name="w", bufs=7))
            epool = ctx.enter_context(tc.tile_pool(name="e", bufs=3))
            scr1 = ctx.enter_context(tc.tile_pool(name="scr1", bufs=1))
            scr2 = ctx.enter_context(tc.tile_pool(name="scr2", bufs=2))
            consts = ctx.enter_context(tc.tile_pool(name="consts", bufs=1))
            lparam = ctx.enter_context(tc.tile_pool(name="lparam", bufs=2))
            ps_mm = ctx.enter_context(tc.tile_pool(name="psmm", bufs=6, space="PSUM"))
            ps_av = ctx.enter_context(tc.tile_pool(name="psav", bufs=2, space="PSUM"))
            avs_p = ctx.enter_context(tc.tile_pool(name="avs", bufs=4))

            # ---- persistent tiles ----
            xt = big.tile([128, NT, S], F32R, tag="x")
            lmem = big.tile([128, NT, M], BF16, tag="lmem")
            rmem = big.tile([128, NT, M], BF16, tag="rmem")
            # act pool: per-sublayer tiles; same-tag allocations reuse the
            # slot serially (ht and at share "ha": never live simultaneously).
            act = ctx.enter_context(tc.tile_pool(name="act", bufs=1))
            kv = ctx.enter_context(tc.tile_pool(name="kv", bufs=3))

            nc.sync.dma_start(out=xt, in_=xT_d.rearrange("(t p) s -> p t s", p=128))
            nc.sync.dma_start(out=lmem, in_=lmemT_d.rearrange("(t p) s -> p t s", p=128))
            nc.sync.dma_start(out=rmem, in_=rmemT_d.rearrange("(t p) s -> p t s", p=128))

            tri = consts.tile([128, 128], BF16)
            nc.sync.dma_start(out=tri, in_=tri_d[:])
            ones_f = consts.tile([128, 64], F32)
            nc.vector.memset(ones_f, 1.0)
            ones_r = consts.tile([128, 1], F32R)
            nc.vector.tensor_copy(out=ones_r, in_=ones_f[:, 0:1])
            zcol = consts.tile([128, 1], F32)
            nc.vector.memset(zcol, 0.0)
            epst = consts.tile([1, 1], F32)
            nc.vector.memset(epst, 1e-5)

            def load_w(dram_ap):
                t = wpool.tile([128, 4, 512], BF16, tag="w")
                nc.sync.dma_start(out=t, in_=dram_ap)
                return t

            def w_slice(dram, l_, q=None):
                # dram [nl, IN, OUT] -> [128, 4, 512] AP
                a = dram[l_].rearrange("(t p) n -> p t n", p=128)
                if q is not None:  # quarter of the free dim
                    a = a[:, :, q * 512:(q + 1) * 512]
                return a

            def emit_ln(x_in, s_ap, b_ap, out_t, out_dtype_is_f32=False):
                """out_t[:, t, :] = (x - mu)/sqrt(var+eps) * s[t] + b[t].
                s_ap/b_ap: [128, NT] SBUF APs."""
                for qh in range(QH):
                    sl = slice(qh * 512, (qh + 1) * 512)
                    sum1 = ps_mm.tile([1, 512], F32, tag="mm", name="sum1")
                    sum2 = ps_mm.tile([1, 512], F32, tag="mm", name="sum2")
                    for k in range(NT):
                        nc.tensor.matmul(sum1, ones_r, x_in[:, k, sl],
                                         start=(k == 0), stop=(k == NT - 1))
                    for k in range(NT):
                        sqt = scr2.tile([128, 512], F32R, tag="sq")
                        nc.vector.tensor_mul(sqt, x_in[:, k, sl], x_in[:, k, sl])
                        nc.tensor.matmul(sum2, ones_r, sqt,
                                         start=(k == 0), stop=(k == NT - 1))
                    mu = scr1.tile([1, 512], F32, tag="mu")
                    nc.vector.tensor_scalar_mul(mu, sum1, 1.0 / D)
                    mm = scr1.tile([1, 512], F32, tag="mm2")
                    nc.vector.tensor_mul(mm, mu, mu)
                    var = scr1.tile([1, 512], F32, tag="var")
                    nc.vector.scalar_tensor_tensor(
                        out=var, in0=sum2, scalar=1.0 / D, in1=mm,
                        op0=ALU.mult, op1=ALU.subtract)
                    rstd = scr1.tile([1, 512], F32, tag="rstd")
                    nc.scalar.activation(rstd, var, AF.Ln, bias=epst)
                    nc.scalar.activation(rstd, rstd, AF.Exp, scale=-0.5)
                    mub = scr1.tile([128, 512], F32, tag="mub")
                    rstdb = scr1.tile([128, 512], F32, tag="rstdb")
                    nc.gpsimd.partition_broadcast(mub, mu)
                    nc.gpsimd.partition_broadcast(rstdb, rstd)
                    for k in range(NT):
                        tmp = scr1.tile([128, 512], F32, tag="lntmp")
                        nc.vector.tensor_sub(tmp, x_in[:, k, sl], mub)
                        nc.vector.tensor_mul(tmp, tmp, rstdb)
                        nc.vector.tensor_scalar(
                            out=out_t[:, k, sl], in0=tmp,
                            scalar1=s_ap[:, k:k + 1], scalar2=b_ap[:, k:k + 1],
                            op0=ALU.mult, op1=ALU.add)

            def emit_projT(w_sb, rhs_t, out_t, bias_sb):
                """out_t[:, m, :] (f32r, [128,NT,S]) = W^T @ rhs + bias.
                w_sb [128,4,512], rhs_t [128,NT,S] f32r, bias_sb [128,NT]."""
                for m_ in range(NT):
                    for qh in range(QH):
                        sl = slice(qh * 512, (qh + 1) * 512)
                        ps = ps_mm.tile([128, 512], F32, tag="mm")
                        for k in range(NT):
                            nc.tensor.matmul(
                                ps, w_sb[:, k, m_ * 128:(m_ + 1) * 128],
                                rhs_t[:, k, sl],
                                start=(k == 0), stop=(k == NT - 1))
                        nc.vector.tensor_scalar(
                            out=out_t[:, m_, sl], in0=ps,
                            scalar1=bias_sb[:, m_:m_ + 1], scalar2=None,
                            op0=ALU.add)

            def emit_v(w_sb, src_t, bvb, vt):
                """vt[:, mt, h, 0:DK] = (src^T)^T @ Wv + bv (natural layout)."""
                nc.vector.tensor_copy(
                    out=vt[:, :, :, DK:DK + 1].rearrange("p a b c -> p (a b c)"),
                    in_=ones_f)
                for mt in range(TT):
                    ps = ps_mm.tile([128, 512], F32, tag="mm")
                    for k in range(NT):
                        nc.tensor.matmul(
                            ps, src_t[:, k, mt * 128:(mt + 1) * 128],
                            w_sb[:, k, :],
                            start=(k == 0), stop=(k == NT - 1))
                    nc.vector.tensor_tensor(
                        out=vt[:, mt, :, 0:DK],
                        in0=ps.rearrange("p (h d) -> p h d", h=H),
                        in1=bvb.rearrange("p (h d) -> p h d", h=H),
                        op=ALU.add)

            def emit_attn(is_self, qt, kt_t, vt, at):
                """at = softmax(k^T q / sqrt(dk)) V, all transposed layouts.
                Heads run in even/odd pairs: their score matmuls use PE row
                groups 0-63 / 64-127 (tile_position auto-derived from the
                lhsT base partition) and execute concurrently."""
                for qh in range(QH):
                    ktmax = (qh + 1) * 4 if is_self else TT
                    for hp in range(H // 2):
                        ets = []
                        for sub in range(2):
                            ets.append(epool.tile([128, TT, 512], BF16, tag="e",
                                                  name=f"et{sub}"))
                        # interleave the two heads' score matmuls (PE row
                        # groups 0-63/64-127 run concurrently)
                        for k in range(ktmax):
                            c0 = max(0, k - qh * 4) * 128 if is_self else 0
                            sps = []
                            for sub in range(2):
                                h_ = 2 * hp + sub
                                po = (h_ % 2) * 64
                                ft_ = h_ // 2
                                sp = ps_mm.tile([128, 512], F32, tag="mm",
                                                name=f"sp{sub}")
                                nc.tensor.matmul(
                                    sp[:, c0:],
                                    kt_t[po:po + 64, ft_, k * 128:(k + 1) * 128],
                                    qt[po:po + 64, ft_, qh * 512 + c0:(qh + 1) * 512],
                                    start=True, stop=True)
                                sps.append(sp)
                            for sub in range(2):
                                nc.scalar.activation(
                                    ets[sub][:, k, c0:], sps[sub][:, c0:], AF.Exp,
                                    scale=1.0 / np.sqrt(DK))
                                if is_self and k >= qh * 4:
                                    nc.gpsimd.tensor_mul(
                                        ets[sub][:, k, c0:c0 + 128],
                                        ets[sub][:, k, c0:c0 + 128], tri)
                        for sub in range(2):
                            h_ = 2 * hp + sub
                            po = (h_ % 2) * 64
                            ft_ = h_ // 2
                            et = ets[sub]
                            av = ps_av.tile([DK + 1, 512], F32, tag="av")
                            for k in range(ktmax):
                                c0 = max(0, k - qh * 4) * 128 if is_self else 0
                                nc.tensor.matmul(
                                    av[:, c0:], vt[:, k, h_, :], et[:, k, c0:],
                                    start=(k == 0), stop=(k == ktmax - 1))
                            # move the AV result to SBUF right away so the
                            # PSUM slot frees; normalize off the critical path
                            sb65 = avs_p.tile([DK + 1, 512], F32, tag="avs")
                            nc.vector.tensor_copy(out=sb65, in_=av)
                            rds = scr1.tile([1, 512], F32, tag="rds")
                            nc.vector.tensor_copy(out=rds, in_=sb65[DK:DK + 1, :])
                            rd = scr1.tile([1, 512], F32, tag="rd")
                            nc.vector.reciprocal_approx_fast(out=rd, in_=rds)
                            rdb = scr2.tile([64, 512], F32, tag="rdb")
                            nc.gpsimd.partition_broadcast(rdb, rd)
                            nc.vector.tensor_mul(
                                at[po:po + 64, ft_, qh * 512:(qh + 1) * 512],
                                sb65[0:DK, :], rdb)

            def emit_resid(w_sb, rhs_t, bias_sb):
                """x += W^T @ rhs + bias (out-projection / FFN-2 path)."""
                for m_ in range(NT):
                    for qh in range(QH):
                        sl = slice(qh * 512, (qh + 1) * 512)
                        ps = ps_mm.tile([128, 512], F32, tag="mm")
                        for k in range(NT):
                            nc.tensor.matmul(
                                ps, w_sb[:, k, m_ * 128:(m_ + 1) * 128],
                                rhs_t[:, k, sl],
                                start=(k == 0), stop=(k == NT - 1))
                        nc.vector.scalar_tensor_tensor(
                            out=xt[:, m_, sl], in0=ps,
                            scalar=bias_sb[:, m_:m_ + 1], in1=xt[:, m_, sl],
                            op0=ALU.add, op1=ALU.add)

            def load_bias_pp(dram, l_):
                t = lparam.tile([128, NT], F32, tag="bpp")
                nc.sync.dma_start(out=t, in_=dram[l_])
                return t

            for l_ in range(nl):
                lns = lparam.tile([128, 4, NT], F32, tag="lns")
                lnb = lparam.tile([128, 4, NT], F32, tag="lnb")
                nc.sync.dma_start(out=lns, in_=lns_d[l_].rearrange("a p t -> p a t"))
                nc.sync.dma_start(out=lnb, in_=lnb_d[l_].rearrange("a p t -> p a t"))

                # -- self QKV --
                ht = act.tile([128, NT, S], BF16, tag="ha")
                emit_ln(xt, lns[:, 0, :], lnb[:, 0, :], ht)
                kvt = {}

                def qkv(a, src_t, with_q):
                    wk = load_w(w_slice(w_d["k" + a], l_))
                    wv = load_w(w_slice(w_d["v" + a], l_))
                    bk = load_bias_pp(b_d["k" + a], l_)
                    bvr = scr1.tile([1, D], F32, tag="bvr")
                    nc.sync.dma_start(out=bvr, in_=b_d["v" + a][l_])
                    bvb = scr1.tile([128, D], F32, tag="bvb")
                    nc.gpsimd.partition_broadcast(bvb, bvr)
                    kt_t = kv.tile([128, NT, S], BF16, tag="kt")
                    vt = kv.tile([128, TT, H, DK + 1], BF16, tag="vt")
                    emit_projT(wk, src_t, kt_t, bk)
                    emit_v(wv, src_t, bvb, vt)
                    kvt[a] = (kt_t, vt)

                def qproj(a, ht_):
                    wq = load_w(w_slice(w_d["q" + a], l_))
                    bq = load_bias_pp(b_d["q" + a], l_)
                    qt = act.tile([128, NT, S], BF16, tag="qt")
                    emit_projT(wq, ht_, qt, bq)
                    return qt

                def oproj(a, at_):
                    wo = load_w(w_slice(w_d["o" + a], l_))
                    bo = load_bias_pp(b_d["o" + a], l_)
                    emit_resid(wo, at_, bo)

                qt = qproj("s", ht)
                qkv("s", ht, True)
                at = act.tile([128, NT, S], BF16, tag="ha")
                emit_attn(True, qt, kvt["s"][0], kvt["s"][1], at)
                oproj("s", at)
                # cross K/V: no dependency on x -- emitted after self-attn so
                # the scheduler uses them to fill PE gaps (LN chains etc.)
                qkv("l", lmem, False)
                qkv("r", rmem, False)

                for si, a in ((1, "l"), (2, "r")):
                    ht = act.tile([128, NT, S], BF16, tag="ha")
                    emit_ln(xt, lns[:, si, :], lnb[:, si, :], ht)
                    qt = qproj(a, ht)
                    at = act.tile([128, NT, S], BF16, tag="ha")
                    emit_attn(False, qt, kvt[a][0], kvt[a][1], at)
                    oproj(a, at)

                # ---- FFN ----
                ht = act.tile([128, NT, S], BF16, tag="ha")
                emit_ln(xt, lns[:, 3, :], lnb[:, 3, :], ht)
                b1 = lparam.tile([128, DFF // 128], F32, tag="b1")
                nc.sync.dma_start(out=b1, in_=b_d["1"][l_])
                b2 = load_bias_pp(b_d["2"], l_)
                for qr in range(4):
                    h1 = act.tile([128, 4, S], BF16, tag="h1")
                    w1 = load_w(w_slice(w_d["1"], l_, q=qr))
                    w2 = load_w(
                        w_d["2"][l_].rearrange("(t p) n -> p t n", p=128)
                        [:, qr * 4:(qr + 1) * 4, :])
                    for dt_ in range(4):
                        for qh in range(QH):
                            sl = slice(qh * 512, (qh + 1) * 512)
                            ps = ps_mm.tile([128, 512], F32, tag="mm")
                            for k in range(NT):
                                nc.tensor.matmul(
                                    ps, w1[:, k, dt_ * 128:(dt_ + 1) * 128],
                                    ht[:, k, sl],
                                    start=(k == 0), stop=(k == NT - 1))
                            nc.scalar.activation(
                                h1[:, dt_, sl], ps, AF.Gelu_apprx_tanh,
                                bias=b1[:, qr * 4 + dt_:qr * 4 + dt_ + 1])
                    for m_ in range(NT):
                        for qh in range(QH):
                            sl = slice(qh * 512, (qh + 1) * 512)
                            ps = ps_mm.tile([128, 512], F32, tag="mm")
                            for dt_ in range(4):
                                nc.tensor.matmul(
                                    ps, w2[:, dt_, m_ * 128:(m_ + 1) * 128],
                                    h1[:, dt_, sl],
                                    start=(dt_ == 0), stop=(dt_ == 3))
                            bsl = b2[:, m_:m_ + 1] if qr == 0 else zcol
                            nc.vector.scalar_tensor_tensor(
                                out=xt[:, m_, sl], in0=ps, scalar=bsl,
                                in1=xt[:, m_, sl], op0=ALU.add, op1=ALU.add)

            # ---- final LN + output ----
            fns = lparam.tile([128, NT], F32, tag="fns")
            fnb = lparam.tile([128, NT], F32, tag="fnb")
            nc.sync.dma_start(out=fns, in_=fns_d[0])
            nc.sync.dma_start(out=fnb, in_=fnb_d[0])
            outt = act.tile([128, NT, S], F32, tag="qt")
            emit_ln(xt, fns, fnb, outt, out_dtype_is_f32=True)
            nc.sync.dma_start(out=out_d.rearrange("(t p) s -> p t s", p=128),
                              in_=outt)

    nc.compile()
    return nc


def _prep_inputs(inputs, num_layers=L):
    """Build per-core in_maps from the full problem inputs."""
    nl = num_layers
    f32 = np.float32
    g = {k: np.asarray(v, dtype=f32) if np.asarray(v).dtype != np.bool_ else v
         for k, v in inputs.items()}

    def pp(a):  # [nl, D] -> [nl, 128, NT] per-partition layout
        return np.ascontiguousarray(
            a[:nl].reshape(nl, NT, 128).transpose(0, 2, 1))

    tri = np.tril(np.ones((128, 128), f32)).T  # tri[p, j] = 1 if p <= j
    common = {}
    for i, a in enumerate(("s", "l", "r")):
        wqkv = g["Wqkv_self" if a == "s" else f"Wqkv_{a}"][:nl]
        bqkv = g["bqkv_self" if a == "s" else f"bqkv_{a}"][:nl]
        wo = g["Wo_self" if a == "s" else f"Wo_{a}"][:nl]
        bo = g["bo_self" if a == "s" else f"bo_{a}"][:nl]
        common[f"wq{a}"] = np.ascontiguousarray(wqkv[:, 0]).astype(ml_dtypes.bfloat16)
        common[f"wk{a}"] = np.ascontiguousarray(wqkv[:, 1]).astype(ml_dtypes.bfloat16)
        common[f"wv{a}"] = np.ascontiguousarray(wqkv[:, 2]).astype(ml_dtypes.bfloat16)
        common[f"wo{a}"] = np.ascontiguousarray(wo).astype(ml_dtypes.bfloat16)
        common[f"bq{a}"] = pp(bqkv[:, 0])
        common[f"bk{a}"] = pp(bqkv[:, 1])
        common[f"bv{a}"] = np.ascontiguousarray(bqkv[:, 2]).reshape(nl, 1, D)
        common[f"bo{a}"] = pp(bo)
    common["w1"] = np.ascontiguousarray(g["W1"][:nl]).astype(ml_dtypes.bfloat16)
    common["w2"] = np.ascontiguousarray(g["W2"][:nl]).astype(ml_dtypes.bfloat16)
    common["b1"] = np.ascontiguousarray(
        g["b1"][:nl].reshape(nl, DFF // 128, 128).transpose(0, 2, 1))
    common["b2"] = pp(g["b2"][:nl])
    common["lns"] = np.ascontiguousarray(
        g["ln_scale"][:nl].reshape(nl, 4, NT, 128).transpose(0, 1, 3, 2))
    common["lnb"] = np.ascontiguousarray(
        g["ln_bias"][:nl].reshape(nl, 4, NT, 128).transpose(0, 1, 3, 2))
    common["fns"] = g["fnorm_scale"].reshape(1, NT, 128).transpose(0, 2, 1).copy()
    common["fnb"] = g["fnorm_bias"].reshape(1, NT, 128).transpose(0, 2, 1).copy()
    common["tri"] = tri.astype(ml_dtypes.bfloat16)

    in_maps = []
    for c in range(8):
        b = c % B
        m = dict(common)
        m["xT"] = np.ascontiguousarray(g["tgt_emb"][b].T)
        m["lmemT"] = np.ascontiguousarray(g["l_mem_emb"][b].T).astype(ml_dtypes.bfloat16)
        m["rmemT"] = np.ascontiguousarray(g["r_mem_emb"][b].T).astype(ml_dtypes.bfloat16)
        in_maps.append(m)
    return in_maps


def run(inputs, num_layers=L, trace=False, tmpdir=None):
    key = num_layers
    if key not in _cache:
        _cache[key] = build_program(num_layers)
    nc = _cache[key]
    in_maps = _prep_inputs(inputs, num_layers)
    res = run_bass_kernel_spmd(nc, in_maps, core_ids=list(range(8)),
                               trace=trace, tmpdir=tmpdir)
    out = np.stack([res.results[b]["out"].T for b in range(B)])
    return out, res


def kernel(**inputs):
    out, _ = run(inputs)
    return out.astype(np.float32)


# revision 20
# speedup vs baseline: 1.5719x; 1.2551x over previous
"""Trainium2 Bass kernel for nn_ASTDecoder (4-layer transformer decoder,
B=4, S=M=1024, D=512, H=8, DFF=2048, fp32).

Sharding: data-parallel over batch. Core c computes batch element c%4 end to
end (cores 4-7 are duplicates whose outputs are ignored).

On-core layout: activations live feature-major ("transposed", [D, tok]) so
every projection is matmul(lhsT=W[D,out], rhs=actT) with natural weights.
Attention computes transposed scores s^T[k,q] = matmul(lhsT=k^T, rhs=q^T);
softmax skips the max-subtraction (scores are O(1) for this model), the
denominator comes for free from a ones-column appended to V, and the causal
mask is applied as a column-range restriction plus one triangular block
multiply per diagonal tile. All matmuls run in float32r.
"""

import sys

sys.path.insert(0, '/opt/trn_rl_repo')

import ml_dtypes
import numpy as np

import concourse.bass as bass  # noqa: F401
import concourse.tile as tile
import concourse.mybir as mybir
from concourse import bacc
from concourse.bass_utils import run_bass_kernel_spmd

F32 = mybir.dt.float32
F32R = mybir.dt.float32r
BF16 = mybir.dt.bfloat16
AF = mybir.ActivationFunctionType
ALU = mybir.AluOpType

B, S, M, D, H, L, DFF = 4, 1024, 1024, 512, 8, 4, 2048
DK = D // H          # 64
NT = D // 128        # 4 feature tiles
TT = S // 128        # 8 token tiles
QH = S // 512        # 2 query halves

_cache = {}

_tables_patched = False


def _patch_act_tables():
    """Route Exp and Ln to the combined natural_log_exp set so LayerNorm's
    Ln->Exp rstd and the attention exps share one ACT table (avoids ~2.7us
    table reloads inside every LayerNorm critical path)."""
    global _tables_patched
    if _tables_patched:
        return
    import concourse.hw_specs as hw_specs
    orig = hw_specs.get_activation_tables

    def patched(module_arch):
        t = {k: set(v) for k, v in orig(module_arch).items()}
        exp = mybir.ActivationFunctionType.Exp
        ln = mybir.ActivationFunctionType.Ln
        for name, funcs in t.items():
            if name != "natural_log_exp_and_others":
                funcs.discard(exp)
                funcs.discard(ln)
        return t

    hw_specs.get_activation_tables = patched
    bacc.get_activation_tables = patched
    _tables_patched = True


def build_program(num_layers=L):
    _patch_act_tables()
    nc = bacc.Bacc("TRN2", target_bir_lowering=False, debug=False, num_devices=8)
    nl = num_layers

    # ---- DRAM I/O ----
    xT_d = nc.dram_tensor("xT", [D, S], F32R, kind="ExternalInput")
    lmemT_d = nc.dram_tensor("lmemT", [D, M], BF16, kind="ExternalInput")
    rmemT_d = nc.dram_tensor("rmemT", [D, M], BF16, kind="ExternalInput")
    tri_d = nc.dram_tensor("tri", [128, 128], BF16, kind="ExternalInput")

    def wt(name, shape):
        return nc.dram_tensor(name, shape, BF16, kind="ExternalInput")

    def ft(name, shape):
        return nc.dram_tensor(name, shape, F32, kind="ExternalInput")

    w_d = {}
    for a in ("s", "l", "r"):
        for m_ in ("q", "k", "v", "o"):
            w_d[m_ + a] = wt(f"w{m_}{a}", [nl, D, D])
    w_d["1"] = wt("w1", [nl, D, DFF])
    w_d["2"] = wt("w2", [nl, DFF, D])

    b_d = {}
    for a in ("s", "l", "r"):
        for m_ in ("q", "k", "o"):
            b_d[m_ + a] = ft(f"b{m_}{a}", [nl, 128, NT])
        b_d["v" + a] = ft(f"bv{a}", [nl, 1, D])
    b_d["1"] = ft("b1", [nl, 128, DFF // 128])
    b_d["2"] = ft("b2", [nl, 128, NT])
    lns_d = ft("lns", [nl, 4, 128, NT])
    lnb_d = ft("lnb", [nl, 4, 128, NT])
    fns_d = ft("fns", [1, 128, NT])
    fnb_d = ft("fnb", [1, 128, NT])

    out_d = nc.dram_tensor("out", [D, S], F32, kind="ExternalOutput")

    with tile.TileContext(nc) as tc:
        import contextlib
        with contextlib.ExitStack() as ctx:
            big = ctx.enter_context(tc.tile_pool(name="big", bufs=1))
            wpool = ctx.enter_context(tc.tile_pool(name="w", bufs=7))
            epool = ctx.enter_context(tc.tile_pool(name="e", bufs=3))
            scr1 = ctx.enter_context(tc.tile_pool(name="scr1", bufs=1))
            scr2 = ctx.enter_context(tc.tile_pool(name="scr2", bufs=2))
            consts = ctx.enter_context(tc.tile_pool(name="consts", bufs=1))
            lparam = ctx.enter_context(tc.tile_pool(name="lparam", bufs=2))
            ps_mm = ctx.enter_context(tc.tile_pool(name="psmm", bufs=6, space="PSUM"))
            ps_av = ctx.enter_context(tc.tile_pool(name="psav", bufs=2, space="PSUM"))
            avs_p = ctx.enter_context(tc.tile_pool(name="avs", bufs=4))

            # ---- persistent tiles ----
            xt = big.tile([128, NT, S], F32R, tag="x")
            lmem = big.tile([128, NT, M], BF16, tag="lmem")
            rmem = big.tile([128, NT, M], BF16, tag="rmem")
            # act pool: per-sublayer tiles; same-tag allocations reuse the
            # slot serially (ht and at share "ha": never live simultaneously).
            act = ctx.enter_context(tc.tile_pool(name="act", bufs=1))
            kv = ctx.enter_context(tc.tile_pool(name="kv", bufs=3))

            nc.sync.dma_start(out=xt, in_=xT_d.rearrange("(t p) s -> p t s", p=128))
            nc.sync.dma_start(out=lmem, in_=lmemT_d.rearrange("(t p) s -> p t s", p=128))
            nc.sync.dma_start(out=rmem, in_=rmemT_d.rearrange("(t p) s -> p t s", p=128))

            tri = consts.tile([128, 128], BF16)
            nc.sync.dma_start(out=tri, in_=tri_d[:])
            ones_f = consts.tile([128, 64], F32)
            nc.vector.memset(ones_f, 1.0)
            ones_r = consts.tile([128, 1], F32R)
            nc.vector.tensor_copy(out=ones_r, in_=ones_f[:, 0:1])
            zcol = consts.tile([128, 1], F32)
            nc.vector.memset(zcol, 0.0)
            epst = consts.tile([1, 1], F32)
            nc.vector.memset(epst, 1e-5)

            def load_w(dram_ap):
                t = wpool.tile([128, 4, 512], BF16, tag="w")
                nc.sync.dma_start(out=t, in_=dram_ap)
                return t

            def w_slice(dram, l_, q=None):
                # dram [nl, IN, OUT] -> [128, 4, 512] AP
                a = dram[l_].rearrange("(t p) n -> p t n", p=128)
                if q is not None:  # quarter of the free dim
                    a = a[:, :, q * 512:(q + 1) * 512]
                return a

            def emit_ln(x_in, s_ap, b_ap, out_t, out_dtype_is_f32=False):
                """out_t[:, t, :] = (x - mu)/sqrt(var+eps) * s[t] + b[t].
                s_ap/b_ap: [128, NT] SBUF APs."""
                for qh in range(QH):
                    sl = slice(qh * 512, (qh + 1) * 512)
                    sum1 = ps_mm.tile([1, 512], F32, tag="mm", name="sum1")
                    sum2 = ps_mm.tile([1, 512], F32, tag="mm", name="sum2")
                    for k in range(NT):
                        nc.tensor.matmul(sum1, ones_r, x_in[:, k, sl],
                                         start=(k == 0), stop=(k == NT - 1))
                    for k in range(NT):
                        sqt = scr2.tile([128, 512], F32R, tag="sq")
                        nc.vector.tensor_mul(sqt, x_in[:, k, sl], x_in[:, k, sl])
                        nc.tensor.matmul(sum2, ones_r, sqt,
                                         start=(k == 0), stop=(k == NT - 1))
                    mu = scr1.tile([1, 512], F32, tag="mu")
                    nc.vector.tensor_scalar_mul(mu, sum1, 1.0 / D)
                    mm = scr1.tile([1, 512], F32, tag="mm2")
                    nc.vector.tensor_mul(mm, mu, mu)
                    var = scr1.tile([1, 512], F32, tag="var")
                    nc.vector.scalar_tensor_tensor(
                        out=var, in0=sum2, scalar=1.0 / D, in1=mm,
                        op0=ALU.mult, op1=ALU.subtract)
                    rstd = scr1.tile([1, 512], F32, tag="rstd")
                    nc.scalar.activation(rstd, var, AF.Ln, bias=epst)
                    nc.scalar.activation(rstd, rstd, AF.Exp, scale=-0.5)
                    mub = scr1.tile([128, 512], F32, tag="mub")
                    rstdb = scr1.tile([128, 512], F32, tag="rstdb")
                    nc.gpsimd.partition_broadcast(mub, mu)
                    nc.gpsimd.partition_broadcast(rstdb, rstd)
                    for k in range(NT):
                        tmp = scr1.tile([128, 512], F32, tag="lntmp")
                        nc.vector.tensor_sub(tmp, x_in[:, k, sl], mub)
                        nc.vector.tensor_mul(tmp, tmp, rstdb)
                        nc.vector.tensor_scalar(
                            out=out_t[:, k, sl], in0=tmp,
                            scalar1=s_ap[:, k:k + 1], scalar2=b_ap[:, k:k + 1],
                            op0=ALU.mult, op1=ALU.add)

            def emit_projT(w_sb, rhs_t, out_t, bias_sb):
                """out_t[:, m, :] (f32r, [128,NT,S]) = W^T @ rhs + bias.
                w_sb [128,4,512], rhs_t [128,NT,S] f32r, bias_sb [128,NT]."""
                for m_ in range(NT):
                    for qh in range(QH):
                        sl = slice(qh * 512, (qh + 1) * 512)
                        ps = ps_mm.tile([128, 512], F32, tag="mm")
                        for k in range(NT):
                            nc.tensor.matmul(
                                ps, w_sb[:, k, m_ * 128:(m_ + 1) * 128],
                                rhs_t[:, k, sl],
                                start=(k == 0), stop=(k == NT - 1))
                        nc.vector.tensor_scalar(
                            out=out_t[:, m_, sl], in0=ps,
                            scalar1=bias_sb[:, m_:m_ + 1], scalar2=None,
                            op0=ALU.add)

            def emit_v(w_sb, src_t, bvb, vt):
                """vt[:, mt, h, 0:DK] = (src^T)^T @ Wv + bv (natural layout)."""
                nc.vector.tensor_copy(
                    out=vt[:, :, :, DK:DK + 1].rearrange("p a b c -> p (a b c)"),
                    in_=ones_f)
                for mt in range(TT):
                    ps = ps_mm.tile([128, 512], F32, tag="mm")
                    for k in range(NT):
                        nc.tensor.matmul(
                            ps, src_t[:, k, mt * 128:(mt + 1) * 128],
                            w_sb[:, k, :],
                            start=(k == 0), stop=(k == NT - 1))
                    nc.vector.tensor_tensor(
                        out=vt[:, mt, :, 0:DK],
                        in0=ps.rearrange("p (h d) -> p h d", h=H),
                        in1=bvb.rearrange("p (h d) -> p h d", h=H),
                        op=ALU.add)

            def emit_attn(is_self, qt, kt_t, vt, at):
                """at = softmax(k^T q / sqrt(dk)) V, all transposed layouts.
                Heads run in even/odd pairs: their score matmuls use PE row
                groups 0-63 / 64-127 (tile_position auto-derived from the
                lhsT base partition) and execute concurrently."""
                for qh in range(QH):
                    ktmax = (qh + 1) * 4 if is_self else TT
                    for hp in range(H // 2):
                        ets = []
                        for sub in range(2):
                            ets.append(epool.tile([128, TT, 512], BF16, tag="e",
                                                  name=f"et{sub}"))
                        # interleave the two heads' score matmuls (PE row
                        # groups 0-63/64-127 run concurrently)
                        for k in range(ktmax):
                            c0 = max(0, k - qh * 4) * 128 if is_self else 0
                            sps = []
                            for sub in range(2):
                                h_ = 2 * hp + sub
                                po = (h_ % 2) * 64
                                ft_ = h_ // 2
                                sp = ps_mm.tile([128, 512], F32, tag="mm",
                                                name=f"sp{sub}")
                                nc.tensor.matmul(
                                    sp[:, c0:],
                                    kt_t[po:po + 64, ft_, k * 128:(k + 1) * 128],
                                    qt[po:po + 64, ft_, qh * 512 + c0:(qh + 1) * 512],
                                    start=True, stop=True)
                                sps.append(sp)
                            for sub in range(2):
                                nc.scalar.activation(
                                    ets[sub][:, k, c0:], sps[sub][:, c0:], AF.Exp,
                                    scale=1.0 / np.sqrt(DK))
                                if is_self and k >= qh * 4:
                                    nc.vector.tensor_mul(
                                        ets[sub][:, k, c0:c0 + 128],
                                        ets[sub][:, k, c0:c0 + 128], tri)
                        for sub in range(2):
                            h_ = 2 * hp + sub
                            po = (h_ % 2) * 64
                            ft_ = h_ // 2
                            et = ets[sub]
                            av = ps_av.tile([DK + 1, 512], F32, tag="av")
                            for k in range(ktmax):
                                c0 = max(0, k - qh * 4) * 128 if is_self else 0
                                nc.tensor.matmul(
                                    av[:, c0:], vt[:, k, h_, :], et[:, k, c0:],
                                    start=(k == 0), stop=(k == ktmax - 1))
                            # move the AV result to SBUF right away so the
                            # PSUM slot frees; normalize off the critical path
                            sb65 = avs_p.tile([DK + 1, 512], F32, tag="avs")
                            nc.vector.tensor_copy(out=sb65, in_=av)
                            rds = scr1.tile([1, 512], F32, tag="rds")
                            nc.vector.tensor_copy(out=rds, in_=sb65[DK:DK + 1, :])
                            rd = scr1.tile([1, 512], F32, tag="rd")
                            nc.vector.reciprocal_approx_fast(out=rd, in_=rds)
                            rdb = scr2.tile([64, 512], F32, tag="rdb")
                            nc.gpsimd.partition_broadcast(rdb, rd)
                            nc.vector.tensor_mul(
                                at[po:po + 64, ft_, qh * 512:(qh + 1) * 512],
                                sb65[0:DK, :], rdb)

            def emit_resid(w_sb, rhs_t, bias_sb):
                """x += W^T @ rhs + bias (out-projection / FFN-2 path)."""
                for m_ in range(NT):
                    for qh in range(QH):
                        sl = slice(qh * 512, (qh + 1) * 512)
                        ps = ps_mm.tile([128, 512], F32, tag="mm")
                        for k in range(NT):
                            nc.tensor.matmul(
                                ps, w_sb[:, k, m_ * 128:(m_ + 1) * 128],
                                rhs_t[:, k, sl],
                                start=(k == 0), stop=(k == NT - 1))
                        nc.vector.scalar_tensor_tensor(
                            out=xt[:, m_, sl], in0=ps,
                            scalar=bias_sb[:, m_:m_ + 1], in1=xt[:, m_, sl],
                            op0=ALU.add, op1=ALU.add)

            def load_bias_pp(dram, l_):
                t = lparam.tile([128, NT], F32, tag="bpp")
                nc.sync.dma_start(out=t, in_=dram[l_])
                return t

            for l_ in range(nl):
                lns = lparam.tile([128, 4, NT], F32, tag="lns")
                lnb = lparam.tile([128, 4, NT], F32, tag="lnb")
                nc.sync.dma_start(out=lns, in_=lns_d[l_].rearrange("a p t -> p a t"))
                nc.sync.dma_start(out=lnb, in_=lnb_d[l_].rearrange("a p t -> p a t"))

                # -- self QKV --
                ht = act.tile([128, NT, S], BF16, tag="ha")
                emit_ln(xt, lns[:, 0, :], lnb[:, 0, :], ht)
                kvt = {}

                def qkv(a, src_t, with_q):
                    wk = load_w(w_slice(w_d["k" + a], l_))
                    wv = load_w(w_slice(w_d["v" + a], l_))
                    bk = load_bias_pp(b_d["k" + a], l_)
                    bvr = scr1.tile([1, D], F32, tag="bvr")
                    nc.sync.dma_start(out=bvr, in_=b_d["v" + a][l_])
                    bvb = scr1.tile([128, D], F32, tag="bvb")
                    nc.gpsimd.partition_broadcast(bvb, bvr)
                    kt_t = kv.tile([128, NT, S], BF16, tag="kt")
                    vt = kv.tile([128, TT, H, DK + 1], BF16, tag="vt")
                    emit_projT(wk, src_t, kt_t, bk)
                    emit_v(wv, src_t, bvb, vt)
                    kvt[a] = (kt_t, vt)

                def qproj(a, ht_):
                    wq = load_w(w_slice(w_d["q" + a], l_))
                    bq = load_bias_pp(b_d["q" + a], l_)
                    qt = act.tile([128, NT, S], BF16, tag="qt")
                    emit_projT(wq, ht_, qt, bq)
                    return qt

                def oproj(a, at_):
                    wo = load_w(w_slice(w_d["o" + a], l_))
                    bo = load_bias_pp(b_d["o" + a], l_)
                    emit_resid(wo, at_, bo)

                qt = qproj("s", ht)
                qkv("s", ht, True)
                at = act.tile([128, NT, S], BF16, tag="ha")
                emit_attn(True, qt, kvt["s"][0], kvt["s"][1], at)
                oproj("s", at)
                # cross K/V: no dependency on x -- emitted after self-attn so
                # the scheduler uses them to fill PE gaps (LN chains etc.)
                qkv("l", lmem, False)
                qkv("r", rmem, False)

                for si, a in ((1, "l"), (2, "r")):
                    ht = act.tile([128, NT, S], BF16, tag="ha")
                    emit_ln(xt, lns[:, si, :], lnb[:, si, :], ht)
                    qt = qproj(a, ht)
                    at = act.tile([128, NT, S], BF16, tag="ha")
                    emit_attn(False, qt, kvt[a][0], kvt[a][1], at)
                    oproj(a, at)

                # ---- FFN ----
                ht = act.tile([128, NT, S], BF16, tag="ha")
                emit_ln(xt, lns[:, 3, :], lnb[:, 3, :], ht)
                b1 = lparam.tile([128, DFF // 128], F32, tag="b1")
                nc.sync.dma_start(out=b1, in_=b_d["1"][l_])
                b2 = load_bias_pp(b_d["2"], l_)
                for qr in range(4):
                    h1 = act.tile([128, 4, S], BF16, tag="h1")
                    w1 = load_w(w_slice(w_d["1"], l_, q=qr))
                    w2 = load_w(
                        w_d["2"][l_].rearrange("(t p) n -> p t n", p=128)
                        [:, qr * 4:(qr + 1) * 4, :])
                    for dt_ in range(4):
                        for qh in range(QH):
                            sl = slice(qh * 512, (qh + 1) * 512)
                            ps = ps_mm.tile([128, 512], F32, tag="mm")
                            for k in range(NT):
                                nc.tensor.matmul(
                                    ps, w1[:, k, dt_ * 128:(dt_ + 1) * 128],
                                    ht[:, k, sl],
                                    start=(k == 0), stop=(k == NT - 1))
                            nc.scalar.activation(
                                h1[:, dt_, sl], ps, AF.Gelu_apprx_tanh,
                                bias=b1[:, qr * 4 + dt_:qr * 4 + dt_ + 1])
                    for m_ in range(NT):
                        for qh in range(QH):
                            sl = slice(qh * 512, (qh + 1) * 512)
                            ps = ps_mm.tile([128, 512], F32, tag="mm")
                            for dt_ in range(4):
                                nc.tensor.matmul(
                                    ps, w2[:, dt_, m_ * 128:(m_ + 1) * 128],
                                    h1[:, dt_, sl],
                                    start=(dt_ == 0), stop=(dt_ == 3))
                            bsl = b2[:, m_:m_ + 1] if qr == 0 else zcol
                            nc.vector.scalar_tensor_tensor(
                                out=xt[:, m_, sl], in0=ps, scalar=bsl,
                                in1=xt[:, m_, sl], op0=ALU.add, op1=ALU.add)

            # ---- final LN + output ----
            fns = lparam.tile([128, NT], F32, tag="fns")
            fnb = lparam.tile([128, NT], F32, tag="fnb")
            nc.sync.dma_start(out=fns, in_=fns_d[0])
            nc.sync.dma_start(out=fnb, in_=fnb_d[0])
            outt = act.tile([128, NT, S], F32, tag="qt")
            emit_ln(xt, fns, fnb, outt, out_dtype_is_f32=True)
            nc.sync.dma_start(out=out_d.rearrange("(t p) s -> p t s", p=128),
                              in_=outt)

    nc.compile()
    return nc


def _prep_inputs(inputs, num_layers=L):
    """Build per-core in_maps from the full problem inputs."""
    nl = num_layers
    f32 = np.float32
    g = {k: np.asarray(v, dtype=f32) if np.asarray(v).dtype != np.bool_ else v
         for k, v in inputs.items()}

    def pp(a):  # [nl, D] -> [nl, 128, NT] per-partition layout
        return np.ascontiguousarray(
            a[:nl].reshape(nl, NT, 128).transpose(0, 2, 1))

    tri = np.tril(np.ones((128, 128), f32)).T  # tri[p, j] = 1 if p <= j
    common = {}
    for i, a in enumerate(("s", "l", "r")):
        wqkv = g["Wqkv_self" if a == "s" else f"Wqkv_{a}"][:nl]
        bqkv = g["bqkv_self" if a == "s" else f"bqkv_{a}"][:nl]
        wo = g["Wo_self" if a == "s" else f"Wo_{a}"][:nl]
        bo = g["bo_self" if a == "s" else f"bo_{a}"][:nl]
        common[f"wq{a}"] = np.ascontiguousarray(wqkv[:, 0]).astype(ml_dtypes.bfloat16)
        common[f"wk{a}"] = np.ascontiguousarray(wqkv[:, 1]).astype(ml_dtypes.bfloat16)
        common[f"wv{a}"] = np.ascontiguousarray(wqkv[:, 2]).astype(ml_dtypes.bfloat16)
        common[f"wo{a}"] = np.ascontiguousarray(wo).astype(ml_dtypes.bfloat16)
        common[f"bq{a}"] = pp(bqkv[:, 0])
        common[f"bk{a}"] = pp(bqkv[:, 1])
        common[f"bv{a}"] = np.ascontiguousarray(bqkv[:, 2]).reshape(nl, 1, D)
        common[f"bo{a}"] = pp(bo)
    common["w1"] = np.ascontiguousarray(g["W1"][:nl]).astype(ml_dtypes.bfloat16)
    common["w2"] = np.ascontiguousarray(g["W2"][:nl]).astype(ml_dtypes.bfloat16)
    common["b1"] = np.ascontiguousarray(
        g["b1"][:nl].reshape(nl, DFF // 128, 128).transpose(0, 2, 1))
    common["b2"] = pp(g["b2"][:nl])
    common["lns"] = np.ascontiguousarray(
        g["ln_scale"][:nl].reshape(nl, 4, NT, 128).transpose(0, 1, 3, 2))
    common["lnb"] = np.ascontiguousarray(
        g["ln_bias"][:nl].reshape(nl, 4, NT, 128).transpose(0, 1, 3, 2))
    common["fns"] = g["fnorm_scale"].reshape(1, NT, 128).transpose(0, 2, 1).copy()
    common["fnb"] = g["fnorm_bias"].reshape(1, NT, 128).transpose(0, 2, 1).copy()
    common["tri"] = tri.astype(ml_dtypes.bfloat16)

    in_maps = []
    for c in range(8):
        b = c % B
        m = dict(common)
        m["xT"] = np.ascontiguousarray(g["tgt_emb"][b].T)
        m["lmemT"] = np.ascontiguousarray(g["l_mem_emb"][b].T).astype(ml_dtypes.bfloat16)
        m["rmemT"] = np.ascontiguousarray(g["r_mem_emb"][b].T).astype(ml_dtypes.bfloat16)
        in_maps.append(m)
    return in_maps


def run(inputs, num_layers=L, trace=False, tmpdir=None):
    key = num_layers
    if key not in _cache:
        _cache[key] = build_program(num_layers)
    nc = _cache[key]
    in_maps = _prep_inputs(inputs, num_layers)
    res = run_bass_kernel_spmd(nc, in_maps, core_ids=list(range(8)),
                               trace=trace, tmpdir=tmpdir)
    out = np.stack([res.results[b]["out"].T for b in range(B)])
    return out, res


def kernel(**inputs):
    out, _ = run(inputs)
    return out.astype(np.float32)
